# revision 30
# baseline (speedup 1.0000x reference)
"""Trainium2 Bass kernel for the soft-logic-gate (moe_routing) problem.

Math (per output column j):
    nw = softmax(weights[j, :16]); W = nw @ COEFF  (4 affine coeffs)
    out[:, j] = W0 + Wa * x[:, idx_a[j]] + Wb * x[:, idx_b[j]] + Wab * a * b

Strategy: shard the 16384 output columns across 8 NeuronCores (2048 each).
Host passes x transposed and quantized to uint8 (xT [in_dim, batch]; x is
uniform in [0,1), so u8 costs ~0.2% rel err) so each output column's source
row is one contiguous 2 KB run; the device gathers rows with indirect DMA,
casting u8->fp16 in the SDMA stream, putting output columns on partitions,
where the per-column coefficients are natural per-partition scalars for
tensor_scalar/activation ops (the 1/255 dequant scales are folded into the
COEFF constants host-side). The result is stored column-major (outT
[shard, batch], contiguous rows) in fp16; the host transposes/upcasts while
unsharding. End-to-end max rel err ~2.9e-3 vs the 2e-2 budget, and HBM reads
for the gathers are 1/4 of the fp32 baseline.

The softmax->coeff fold runs on device in fp32: W_k = (exp(w) @ COEFF_k) /
sum_g exp(w_g), computed for all 16 column-blocks in a handful of [128, 256]
vector ops.  Compute per block: t = Wab*a + Wb, v = Wa*a + W0 (split between
the scalar and vector engines, alternating per block to balance load),
u = b*t and r = u + v on the vector engine.
"""

import json
from contextlib import ExitStack

import numpy as np

from concourse import bass, mybir, tile


def _split_multiwait_bir(bir_json: bytes) -> bytes:
    """Rewrite BIR so no compute instruction carries more than one sem wait.

    The walrus build in this container rejects >1 embedded sync wait per
    compute instruction ("Too many sync wait commands"), but the Tile
    scheduler emits them. Hoist extra waits onto standalone EventSemaphore
    instructions inserted immediately before, on the same engine (waits are
    AND conditions, so splitting preserves semantics and per-engine order).
    """
    b = json.loads(bir_json)
    counter = 0
    for f in b.get("functions", []):
        for blk in f.get("blocks", []):
            ins = blk.get("instructions")
            if not ins:
                continue
            out = []
            for inst in ins:
                si = inst.get("sync_info") or {}
                waits = si.get("on_wait") or []
                if len(waits) > 1 and "engine" in inst:
                    for w in waits[:-1]:
                        counter += 1
                        out.append(
                            {
                                "debug": inst.get("debug", 0),
                                "engine": inst["engine"],
                                "ins": [],
                                "outs": [],
                                "name": f"evw_{counter}_{inst['name']}",
                                "opcode": "EventSemaphore",
                                "sync_info": {"on_update": [], "on_wait": [w]},
                            }
                        )
                    si["on_wait"] = [waits[-1]]
                out.append(inst)
            blk["instructions"] = out
    return json.dumps(b).encode()


def _install_bir_patch():
    import concourse.bass_utils as _bu
    import concourse.bass2jax as _b2j

    if getattr(_bu, "_multiwait_patch", False):
        return
    orig = _bu.compile_bir_kernel

    def patched(bir_json, tmpdir, neff_name="file.neff"):
        return orig(_split_multiwait_bir(bir_json), tmpdir, neff_name)

    _bu.compile_bir_kernel = patched
    _b2j.compile_bir_kernel = patched
    _bu._multiwait_patch = True


_install_bir_patch()

F32 = mybir.dt.float32
F16 = mybir.dt.float16
I32 = mybir.dt.int32
P = 128

# COEFF[g] = [k0, ka, kb, kab]: gate_g(a,b) = k0 + ka*a + kb*b + kab*a*b
COEFF = np.array(
    [
        [0, 0, 0, 0], [0, 0, 0, 1], [0, 1, 0, -1], [0, 1, 0, 0],
        [0, 0, 1, -1], [0, 0, 1, 0], [0, 1, 1, -2], [0, 1, 1, -1],
        [1, -1, -1, 1], [1, -1, -1, 2], [1, 0, -1, 0], [1, 0, -1, 1],
        [1, -1, 0, 0], [1, -1, 0, 1], [1, 0, 0, -1], [1, 0, 0, 0],
    ],
    dtype=np.float32,
)  # [16 gates, 4 coeffs]

NG = 16  # number of gates


def build_nc(B, IN, SH, num_devices=8):
    """Build the per-core Bass program.

    B: batch size, IN: in_dim, SH: output-column shard per core.
    DRAM tensors (per core): xT [IN, B] fp16 (replicated), wq [P, nblk*NG]
    (wq[p, blk*NG+g] = weights[blk*P+p, g]), idxa/idxb [P, nblk] int32
    (idx*[p, blk] = shard index blk*P+p), cfr [P, 4*nblk*NG]
    (cfr[p, k*nblk*NG + blk*NG + g] = COEFF[g, k]), outT [SH, B] fp16.
    """
    nblk = SH // P
    assert SH % P == 0 and B % P == 0

    nc = bass.Bass("TRN2", debug=False, num_devices=num_devices,
                  dynamic_dma_scratch_size=65536)
    xT = nc.dram_tensor("xT", [IN, B], mybir.dt.uint8, kind="ExternalInput").ap()
    # idx[:, :nblk] = idx_a, idx[:, nblk:] = idx_b (one load, gathers first)
    idx = nc.dram_tensor("idx", [P, 2 * nblk], I32, kind="ExternalInput").ap()
    # wc[:, :nblk*NG] = wq, wc[:, nblk*NG:] = cfr
    wc = nc.dram_tensor("wc", [P, 5 * nblk * NG], F32, kind="ExternalInput").ap()
    outT = nc.dram_tensor("outT", [SH, B], F16, kind="ExternalOutput").ap()

    mult, add = mybir.AluOpType.mult, mybir.AluOpType.add
    AF = mybir.ActivationFunctionType

    with tile.TileContext(nc) as tc, ExitStack() as ctx:
        consts = ctx.enter_context(tc.tile_pool(name="consts", bufs=1))
        wpool = ctx.enter_context(tc.tile_pool(name="w", bufs=1))
        gpool = ctx.enter_context(tc.tile_pool(name="gather", bufs=8))
        tpool = ctx.enter_context(tc.tile_pool(name="temps", bufs=4))
        opool = ctx.enter_context(tc.tile_pool(name="outs", bufs=4))

        idx_t = consts.tile([P, 2 * nblk], I32)
        nc.sync.dma_start(idx_t[:], idx)
        wc_t = consts.tile([P, 5 * nblk * NG], F32)
        nc.sync.dma_start(wc_t[:], wc)

        # Per-column effective coefficients W [P, 4*nblk]; W[:, k*nblk+blk]
        # is coeff k for output columns j = blk*P + p.
        W = consts.tile([P, 4 * nblk], F32)
        esum = consts.tile([P, nblk], F32)
        rsum = consts.tile([P, nblk], F32)

        e = wpool.tile([P, nblk * NG], F32)
        nc.scalar.activation(e[:], wc_t[:, :nblk * NG], AF.Exp)
        e3 = e[:].rearrange("p (n g) -> p n g", g=NG)
        nc.vector.reduce_sum(out=esum[:], in_=e3, axis=mybir.AxisListType.X)
        nc.vector.reciprocal(rsum[:], esum[:])
        for k in range(4):
            scr = wpool.tile([P, nblk * NG], F32, tag="scr")
            nc.vector.tensor_tensor(
                out=scr[:],
                in0=e[:],
                in1=wc_t[:, (1 + k) * nblk * NG:(2 + k) * nblk * NG],
                op=mult,
            )
            nc.vector.reduce_sum(
                out=W[:, k * nblk:(k + 1) * nblk],
                in_=scr[:].rearrange("p (n g) -> p n g", g=NG),
                axis=mybir.AxisListType.X,
            )
            nc.vector.tensor_tensor(
                out=W[:, k * nblk:(k + 1) * nblk],
                in0=W[:, k * nblk:(k + 1) * nblk],
                in1=rsum[:],
                op=mult,
            )

        for blk in range(nblk):
            a_t = gpool.tile([P, B], F16, tag="a")
            nc.gpsimd.indirect_dma_start(
                out=a_t[:],
                out_offset=None,
                in_=xT,
                in_offset=bass.IndirectOffsetOnAxis(
                    ap=idx_t[:, blk:blk + 1], axis=0
                ),
            )
            b_t = gpool.tile([P, B], F16, tag="b")
            nc.gpsimd.indirect_dma_start(
                out=b_t[:],
                out_offset=None,
                in_=xT,
                in_offset=bass.IndirectOffsetOnAxis(
                    ap=idx_t[:, nblk + blk:nblk + blk + 1], axis=0
                ),
            )
            W0 = W[:, 0 * nblk + blk: 0 * nblk + blk + 1]
            Wa = W[:, 1 * nblk + blk: 1 * nblk + blk + 1]
            Wb = W[:, 2 * nblk + blk: 2 * nblk + blk + 1]
            Wab = W[:, 3 * nblk + blk: 3 * nblk + blk + 1]

            # r = (Wa*a + W0) + b*(Wab*a + Wb); alternate which affine runs
            # on the scalar engine to balance ACT vs DVE load. The final
            # block is processed in half-width chunks so its first store
            # starts (and the kernel tail ends) sooner.
            t_t = tpool.tile([P, B], F16, tag="t")
            v_t = tpool.tile([P, B], F16, tag="v")
            u_t = tpool.tile([P, B], F16, tag="u")
            r_t = opool.tile([P, B], F16, tag="r")
            chunks = [slice(0, B)] if blk < nblk - 1 else [
                slice(0, B // 2), slice(B // 2, B)
            ]
            for cs in chunks:
                a_c, b_c = a_t[:, cs], b_t[:, cs]
                t_c, v_c, u_c, r_c = t_t[:, cs], v_t[:, cs], u_t[:, cs], r_t[:, cs]
                if blk % 2 == 0:
                    nc.scalar.activation(t_c, a_c, AF.Identity, bias=Wb, scale=Wab)
                    nc.scalar.activation(v_c, a_c, AF.Identity, bias=W0, scale=Wa)
                else:
                    nc.vector.tensor_scalar(
                        out=t_c, in0=a_c, scalar1=Wab, scalar2=Wb,
                        op0=mult, op1=add,
                    )
                    nc.scalar.activation(v_c, a_c, AF.Identity, bias=W0, scale=Wa)
                nc.vector.tensor_tensor(out=u_c, in0=b_c, in1=t_c, op=mult)
                nc.vector.tensor_tensor(out=r_c, in0=u_c, in1=v_c, op=add)
                nc.sync.dma_start(outT[blk * P:(blk + 1) * P, cs], r_t[:, cs])
    return nc


def make_core_inputs(x, weights, idx_a, idx_b, n_cores):
    """Host-side shard prep. Returns (in_maps, SH)."""
    B, IN = x.shape
    OUT = weights.shape[0]
    SH = OUT // n_cores
    nblk = SH // P
    # x in [0,1) is shipped as uint8 (the SWDGE gather casts u8->fp16 on the
    # fly); the 1/255 dequant scales are folded into the COEFF constants:
    # a = a8/255 etc., so ka,kb scale by 1/255 and kab by 1/255^2.
    xT = np.ascontiguousarray(
        np.rint(np.asarray(x, dtype=np.float32).T * 255.0).astype(np.uint8)
    )
    cq = COEFF.copy()
    cq[:, 1] /= 255.0
    cq[:, 2] /= 255.0
    cq[:, 3] /= 255.0 * 255.0
    # cfr[p, k*nblk*NG + blk*NG + g] = cq[g, k], replicated over p and blk
    cfr = np.ascontiguousarray(
        np.broadcast_to(
            np.repeat(cq.T.reshape(4, 1, NG), nblk, axis=1).reshape(
                1, 4 * nblk * NG
            ),
            (P, 4 * nblk * NG),
        )
    ).astype(np.float32)
    idx_a = np.asarray(idx_a)
    idx_b = np.asarray(idx_b)
    weights = np.asarray(weights, dtype=np.float32)
    in_maps = []
    for c in range(n_cores):
        sl = slice(c * SH, (c + 1) * SH)
        ia = idx_a[sl].reshape(nblk, P).T.astype(np.int32)
        ib = idx_b[sl].reshape(nblk, P).T.astype(np.int32)
        idxc = np.ascontiguousarray(np.concatenate([ia, ib], axis=1))
        # wq[p, blk*NG+g] = weights[c*SH + blk*P + p, g]
        wqc = weights[sl].reshape(nblk, P, NG).transpose(1, 0, 2).reshape(
            P, nblk * NG
        )
        wcc = np.ascontiguousarray(np.concatenate([wqc, cfr], axis=1))
        in_maps.append({"xT": xT, "wc": wcc, "idx": idxc})
    return in_maps, SH


_CACHE = {}

N_CORES = 8


def _get_nc(B, IN, SH):
    key = (B, IN, SH)
    if key not in _CACHE:
        _CACHE[key] = build_nc(B, IN, SH, num_devices=N_CORES)
    return _CACHE[key]


def kernel(x, weights, idx_a, idx_b, _trace=False, _tmpdir=None):
    """Full-input entry point: returns out [batch, out_dim] float32."""
    from concourse.bass_utils import run_bass_kernel_spmd

    x = np.asarray(x, dtype=np.float32)
    B, IN = x.shape
    in_maps, SH = make_core_inputs(x, weights, idx_a, idx_b, N_CORES)
    nc = _get_nc(B, IN, SH)
    kw = {}
    if _trace:
        kw = {"trace": True, "tmpdir": _tmpdir}
    res = run_bass_kernel_spmd(nc, in_maps, core_ids=list(range(N_CORES)), **kw)
    full = np.empty((B, SH * N_CORES), dtype=np.float32)
    for c in range(N_CORES):
        full[:, c * SH:(c + 1) * SH] = res.results[c]["outT"].T.astype(np.float32)
    if _trace:
        kernel.last_results = res
    return full
